# revision 1
# baseline (speedup 1.0000x reference)
"""Trainium2 Bass kernel for nn_CMFA (dense_transformer, seq_len=1 cross-attention).

Math notes (exact simplifications vs the reference):
  - softmax over a single key is exactly 1.0, so the attention output is
    exactly the v-projection: mha(q,k,v) = (v @ Wv.T + bv) @ Wo.T + bo.
    The q/k projections never influence the output.
  - Wv -> Wo -> fi2 is a linear chain (no nonlinearity), so it is folded on
    the host:  V = [v1, i_] @ Wcat.T + bcat  with
      Wcat = [fi2 @ (Wo @ Wv), fi2],  bcat = fi2 @ (Wo @ bv + bo) + fi2_b
    (the i_ column block carries the residual through fi2).

Device layout: activations are feature-major ("transposed", [feat, batch]) so
every matmul contracts over the partition dim and every DMA is contiguous.
The host pre-transposes the batch shards of i/t and transposes the output
back. Pure data parallel across 8 cores; weights replicated.

Per-(layer, k-chunk) weight tiles give exact DMA->matmul dependencies, so
the PE starts as soon as the first 256KB chunks land. Input loads for batch
tile n+1 are emitted right after tile n's fi1 matmuls (with a 16-slot x
pool) so the in-order Sync dispatch queue prefetches them ahead of tile n's
output stores.
"""

import numpy as np

B, IMG, TAB, HID = 32768, 2048, 128, 512
NCORES = 8
BS = B // NCORES  # rows per core
NT = 512          # batch-tile (matmul moving/free dim)

_CACHE = {}


def _pack_blocks(WT: np.ndarray, K: int, M: int) -> np.ndarray:
    """[K*128, M*128] -> [128, K*M*128] with col ((k*M+m)*128 + j) = WT[k*128+p, m*128+j]."""
    out = WT.reshape(K, 128, M, 128).transpose(1, 0, 2, 3).reshape(128, K * M * 128)
    return np.ascontiguousarray(out, dtype=np.float32)


def _build_nc(bs: int):
    import concourse.bass as bass
    import concourse.tile as tile
    from concourse import bacc, mybir

    f32 = mybir.dt.float32
    f32r = mybir.dt.float32r
    Relu = mybir.ActivationFunctionType.Relu
    Ident = mybir.ActivationFunctionType.Identity
    ntiles = bs // NT

    nc = bacc.Bacc("TRN2", target_bir_lowering=False, debug=False)

    iT_d = nc.dram_tensor("iT", [IMG, bs], f32r, kind="ExternalInput").ap()
    tT_d = nc.dram_tensor("tT", [TAB, bs], f32r, kind="ExternalInput").ap()
    w_fi1_d = nc.dram_tensor("w_fi1", [128, 64 * 128], f32r, kind="ExternalInput").ap()
    w_ft1_d = nc.dram_tensor("w_ft1", [128, 4 * 128], f32r, kind="ExternalInput").ap()
    w_ci1_d = nc.dram_tensor("w_ci1", [128, 16 * 128], f32r, kind="ExternalInput").ap()
    w_ct1_d = nc.dram_tensor("w_ct1", [128, 16 * 128], f32r, kind="ExternalInput").ap()
    w_V_d = nc.dram_tensor("w_V", [128, 32 * 128], f32r, kind="ExternalInput").ap()
    w_T_d = nc.dram_tensor("w_T", [128, 32 * 128], f32r, kind="ExternalInput").ap()
    bias_d = nc.dram_tensor("bias", [128, 24], f32, kind="ExternalInput").ap()
    out_d = nc.dram_tensor("outT", [2 * HID, bs], f32, kind="ExternalOutput").ap()

    with tile.TileContext(nc) as tc:
        with (
            tc.tile_pool(name="w", bufs=1) as wpool,
            tc.tile_pool(name="x", bufs=16) as xpool,
            tc.tile_pool(name="h", bufs=6) as hpool,
            tc.tile_pool(name="o", bufs=8) as opool,
            tc.tile_pool(name="ps", bufs=8, space="PSUM") as pspool,
        ):
            def wchunks(K, lname):
                return [wpool.tile([128, 4 * 128], f32r, name=f"w_{lname}_{k}")
                        for k in range(K)]

            wf1 = wchunks(16, "fi1")
            wt1 = wchunks(1, "ft1")
            wc1 = wchunks(4, "ci1")
            wc2 = wchunks(4, "ct1")
            wV = wchunks(8, "V")
            wT = wchunks(8, "T")
            bt = wpool.tile([128, 24], f32, name="bias_t")

            def xload(n):
                xs = []
                c0 = n * NT
                for k in range(16):
                    xk = xpool.tile([128, NT], f32r, tag="x", name=f"xk_{n}_{k}")
                    nc.sync.dma_start(xk[:], iT_d[128 * k:128 * (k + 1), c0:c0 + NT])
                    xs.append(xk)
                return xs

            # preamble: first tile's x chunks interleaved with fi1 weight chunks
            x_cur = [xpool.tile([128, NT], f32r, tag="x", name=f"xk_0_{k}")
                     for k in range(16)]
            nc.sync.dma_start(bt[:], bias_d[:])
            for k in range(16):
                nc.sync.dma_start(x_cur[k][:], iT_d[128 * k:128 * (k + 1), 0:NT])
                nc.sync.dma_start(wf1[k][:], w_fi1_d[:, 512 * k:512 * (k + 1)])
            xt_cur = xpool.tile([128, NT], f32r, tag="xt", bufs=2, name="xt_0")
            nc.sync.dma_start(xt_cur[:], tT_d[:, 0:NT])
            for tiles, dram in [(wt1, w_ft1_d), (wc1, w_ci1_d), (wc2, w_ct1_d),
                                (wV, w_V_d), (wT, w_T_d)]:
                for j, wtile in enumerate(tiles):
                    nc.sync.dma_start(wtile[:], dram[:, 512 * j:512 * (j + 1)])

            def mm(ps_ap, wtiles, k, m, x_ap, start, stop):
                nc.tensor.matmul(
                    ps_ap,
                    wtiles[k][:, m * 128:(m + 1) * 128],
                    x_ap,
                    start=start,
                    stop=stop,
                )

            for n in range(ntiles):
                c0 = n * NT
                # ---- i_ = relu(i @ fi1.T + b) ----
                ps1 = [pspool.tile([128, NT], f32, tag="ps", name=f"ps1_{n}_{_m}") for _m in range(4)]
                for k in range(16):
                    for m in range(4):
                        mm(ps1[m][:], wf1, k, m, x_cur[k][:], k == 0, k == 15)

                # prefetch next tile's inputs (early in Sync program order)
                if n + 1 < ntiles:
                    x_nxt = xload(n + 1)
                    xt_nxt = xpool.tile([128, NT], f32r, tag="xt", bufs=2,
                                        name=f"xt_{n + 1}")
                    nc.sync.dma_start(xt_nxt[:], tT_d[:, c0 + NT:c0 + 2 * NT])

                i_ = [hpool.tile([128, NT], f32r, tag="i_", name=f"i__{n}_{_m}") for _m in range(4)]
                for m in range(4):
                    nc.scalar.activation(i_[m][:], ps1[m][:], Relu, bias=bt[:, m:m + 1])

                # ---- t_ = relu(t @ ft1.T + b) ----
                ps2 = [pspool.tile([128, NT], f32, tag="ps", name=f"ps2_{n}_{_m}") for _m in range(4)]
                for m in range(4):
                    mm(ps2[m][:], wt1, 0, m, xt_cur[:], True, True)
                t_ = [hpool.tile([128, NT], f32r, tag="t_", name=f"t__{n}_{_m}") for _m in range(4)]
                for m in range(4):
                    nc.scalar.activation(t_[m][:], ps2[m][:], Relu, bias=bt[:, 4 + m:5 + m])

                # ---- v1 = relu(i_ @ ci1.T + b) ----
                ps3 = [pspool.tile([128, NT], f32, tag="ps", name=f"ps3_{n}_{_m}") for _m in range(4)]
                for k in range(4):
                    for m in range(4):
                        mm(ps3[m][:], wc1, k, m, i_[k][:], k == 0, k == 3)
                v1 = [hpool.tile([128, NT], f32r, tag="v1", name=f"v1_{n}_{_m}") for _m in range(4)]
                for m in range(4):
                    nc.scalar.activation(v1[m][:], ps3[m][:], Relu, bias=bt[:, 8 + m:9 + m])

                # ---- v2 = relu(t_ @ ct1.T + b) ----
                ps4 = [pspool.tile([128, NT], f32, tag="ps", name=f"ps4_{n}_{_m}") for _m in range(4)]
                for k in range(4):
                    for m in range(4):
                        mm(ps4[m][:], wc2, k, m, t_[k][:], k == 0, k == 3)
                v2 = [hpool.tile([128, NT], f32r, tag="v2", name=f"v2_{n}_{_m}") for _m in range(4)]
                for m in range(4):
                    nc.scalar.activation(v2[m][:], ps4[m][:], Relu, bias=bt[:, 12 + m:13 + m])

                # ---- V = [v1, i_] @ WcatV.T + bcatV ----
                psV = [pspool.tile([128, NT], f32, tag="ps", name=f"psV_{n}_{_m}") for _m in range(4)]
                for k in range(4):
                    for m in range(4):
                        mm(psV[m][:], wV, k, m, v1[k][:], k == 0, False)
                for k in range(4):
                    for m in range(4):
                        mm(psV[m][:], wV, 4 + k, m, i_[k][:], False, k == 3)
                for m in range(4):
                    oV = opool.tile([128, NT], f32, tag="o", name=f"oV_{n}_{m}")
                    nc.scalar.activation(oV[:], psV[m][:], Ident, bias=bt[:, 16 + m:17 + m])
                    nc.sync.dma_start(out_d[128 * m:128 * (m + 1), c0:c0 + NT], oV[:])

                # ---- T = [v2, t_] @ WcatT.T + bcatT ----
                psT = [pspool.tile([128, NT], f32, tag="ps", name=f"psT_{n}_{_m}") for _m in range(4)]
                for k in range(4):
                    for m in range(4):
                        mm(psT[m][:], wT, k, m, v2[k][:], k == 0, False)
                for k in range(4):
                    for m in range(4):
                        mm(psT[m][:], wT, 4 + k, m, t_[k][:], False, k == 3)
                for m in range(4):
                    oT = opool.tile([128, NT], f32, tag="o", name=f"oT_{n}_{m}")
                    nc.scalar.activation(oT[:], psT[m][:], Ident, bias=bt[:, 20 + m:21 + m])
                    nc.sync.dma_start(
                        out_d[HID + 128 * m:HID + 128 * (m + 1), c0:c0 + NT], oT[:]
                    )

                if n + 1 < ntiles:
                    x_cur = x_nxt
                    xt_cur = xt_nxt

    nc.compile()
    return nc


def _host_pack(inp: dict):
    f8 = np.float64
    fi1_w, fi1_b = inp["fi1_w"], inp["fi1_b"]
    ft1_w, ft1_b = inp["ft1_w"], inp["ft1_b"]
    ci1_w, ci1_b = inp["ci1_w"], inp["ci1_b"]
    ct1_w, ct1_b = inp["ct1_w"], inp["ct1_b"]

    def fold(wv, bv, wo, bo, f_w, f_b):
        Wvo = wo.astype(f8) @ wv.astype(f8)
        bvo = wo.astype(f8) @ bv.astype(f8) + bo.astype(f8)
        Wcat = np.concatenate([f_w.astype(f8) @ Wvo, f_w.astype(f8)], axis=1)
        bcat = f_w.astype(f8) @ bvo + f_b.astype(f8)
        return Wcat.astype(np.float32), bcat.astype(np.float32)

    WcatV, bcatV = fold(inp["aV_wv"], inp["aV_bv"], inp["aV_wo"], inp["aV_bo"],
                        inp["fi2_w"], inp["fi2_b"])
    WcatT, bcatT = fold(inp["aT_wv"], inp["aT_bv"], inp["aT_wo"], inp["aT_bo"],
                        inp["ft2_w"], inp["ft2_b"])

    weights = {
        "w_fi1": _pack_blocks(np.ascontiguousarray(fi1_w.T), 16, 4),
        "w_ft1": _pack_blocks(np.ascontiguousarray(ft1_w.T), 1, 4),
        "w_ci1": _pack_blocks(np.ascontiguousarray(ci1_w.T), 4, 4),
        "w_ct1": _pack_blocks(np.ascontiguousarray(ct1_w.T), 4, 4),
        "w_V": _pack_blocks(np.ascontiguousarray(WcatV.T), 8, 4),
        "w_T": _pack_blocks(np.ascontiguousarray(WcatT.T), 8, 4),
    }
    cols = []
    for b in (fi1_b, ft1_b, ci1_b, ct1_b, bcatV, bcatT):
        for m in range(4):
            cols.append(b[128 * m:128 * (m + 1)])
    weights["bias"] = np.ascontiguousarray(np.stack(cols, axis=1), dtype=np.float32)
    return weights


def kernel(**inputs) -> np.ndarray:
    from concourse import bass_utils

    i = np.asarray(inputs["i"], dtype=np.float32)
    t = np.asarray(inputs["t"], dtype=np.float32)
    weights = _host_pack(inputs)

    if "nc" not in _CACHE:
        _CACHE["nc"] = _build_nc(BS)
    nc = _CACHE["nc"]

    in_maps = []
    for c in range(NCORES):
        sl = slice(c * BS, (c + 1) * BS)
        m = dict(weights)
        m["iT"] = np.ascontiguousarray(i[sl].T)
        m["tT"] = np.ascontiguousarray(t[sl].T)
        in_maps.append(m)

    res = bass_utils.run_bass_kernel_spmd(nc, in_maps, core_ids=list(range(NCORES)))

    out = np.empty((B, 2 * HID), dtype=np.float32)
    for c in range(NCORES):
        out[c * BS:(c + 1) * BS] = res.results[c]["outT"].T
    return out



# revision 4
# speedup vs baseline: 1.0027x; 1.0027x over previous
"""Trainium2 Bass kernel for nn_CMFA (dense_transformer, seq_len=1 cross-attention).

Math notes (exact simplifications vs the reference):
  - softmax over a single key is exactly 1.0, so the attention output is
    exactly the v-projection: mha(q,k,v) = (v @ Wv.T + bv) @ Wo.T + bo.
    The q/k projections never influence the output.
  - Wv -> Wo -> fi2 is a linear chain (no nonlinearity), so it is folded on
    the host:  V = [v1, i_] @ Wcat.T + bcat  with
      Wcat = [fi2 @ (Wo @ Wv), fi2],  bcat = fi2 @ (Wo @ bv + bo) + fi2_b
    (the i_ column block carries the residual through fi2).

Precision: matmul operands (inputs, weights, intermediate activations) are
fp16; PSUM accumulation, biases and the final output are fp32. fp16 keeps
the PE at 1 cycle/row (same as f32r at 512-wide moving dim) but halves the
stationary-weight load so it hides completely under the previous matmul's
streaming window, and halves HBM traffic. Measured end-to-end error vs the
f32 reference is ~5e-4 (gate is 2e-2).

Device layout: activations are feature-major ("transposed", [feat, batch]) so
every matmul contracts over the partition dim and every DMA is contiguous.
The host pre-transposes the batch shards of i/t and transposes the output
back. Pure data parallel across 8 cores; weights replicated.

Schedule: m-outer / k-inner everywhere so each PSUM bank closes as early as
possible and the activation engine drains it while the PE works on the next
output block. x for batch-tile n+1 is prefetched with 2 combined DMAs on the
(otherwise idle) gpsimd queue; output stores go on the vector queue; the
(otherwise idle) gpsimd queue along with the output stores; the sync queue
only carries the preamble weight/x0 loads that gate startup.
"""

import numpy as np

B, IMG, TAB, HID = 32768, 2048, 128, 512
NCORES = 8
BS = B // NCORES  # rows per core
NT = 512          # batch-tile (matmul moving/free dim)
KI = IMG // 128   # 16 contraction chunks for fi1

_CACHE = {}


def _pack_blocks(WT: np.ndarray, K: int, M: int) -> np.ndarray:
    """[K*128, M*128] -> [128, K*M*128] with col ((k*M+m)*128 + j) = WT[k*128+p, m*128+j]."""
    out = WT.reshape(K, 128, M, 128).transpose(1, 0, 2, 3).reshape(128, K * M * 128)
    return np.ascontiguousarray(out, dtype=np.float16)


def _build_nc(bs: int):
    import concourse.bass as bass
    import concourse.tile as tile
    from concourse import bacc, mybir

    f32 = mybir.dt.float32
    f16 = mybir.dt.float16
    Relu = mybir.ActivationFunctionType.Relu
    Ident = mybir.ActivationFunctionType.Identity
    ntiles = bs // NT

    nc = bacc.Bacc("TRN2", target_bir_lowering=False, debug=False)

    iT_d = nc.dram_tensor("iT", [128, KI, bs], f16, kind="ExternalInput").ap()
    tT_d = nc.dram_tensor("tT", [TAB, bs], f16, kind="ExternalInput").ap()
    w_fi1_d = nc.dram_tensor("w_fi1", [128, 64 * 128], f16, kind="ExternalInput").ap()
    w_ft1_d = nc.dram_tensor("w_ft1", [128, 4 * 128], f16, kind="ExternalInput").ap()
    w_ci1_d = nc.dram_tensor("w_ci1", [128, 16 * 128], f16, kind="ExternalInput").ap()
    w_ct1_d = nc.dram_tensor("w_ct1", [128, 16 * 128], f16, kind="ExternalInput").ap()
    w_V_d = nc.dram_tensor("w_V", [128, 32 * 128], f16, kind="ExternalInput").ap()
    w_T_d = nc.dram_tensor("w_T", [128, 32 * 128], f16, kind="ExternalInput").ap()
    bias_d = nc.dram_tensor("bias", [128, 24], f32, kind="ExternalInput").ap()
    out_d = nc.dram_tensor("outT", [2 * HID, bs], f32, kind="ExternalOutput").ap()

    with tile.TileContext(nc) as tc:
        with (
            tc.tile_pool(name="w", bufs=1) as wpool,
            tc.tile_pool(name="x", bufs=3) as xpool,
            tc.tile_pool(name="h", bufs=8) as hpool,
            tc.tile_pool(name="o", bufs=8) as opool,
            tc.tile_pool(name="ps", bufs=8, space="PSUM") as pspool,
        ):
            def wchunks(K, lname):
                return [wpool.tile([128, 4 * 128], f16, name=f"w_{lname}_{k}")
                        for k in range(K)]

            wf1 = wchunks(KI, "fi1")
            wt1 = wchunks(1, "ft1")
            wc1 = wchunks(4, "ci1")
            wc2 = wchunks(4, "ct1")
            wV = wchunks(8, "V")
            wT = wchunks(8, "T")
            bt = wpool.tile([128, 24], f32, name="bias_t")

            # preamble: first-tile x chunks interleaved with fi1 weight chunks,
            # the startup-critical pair (x chunk 0, wf1[0]) first.
            x_cur = xpool.tile([128, KI, NT], f16, tag="x", name="x_0")
            xt_cur = xpool.tile([128, NT], f16, tag="xt", name="xt_0")
            nc.sync.dma_start(x_cur[:, 0, :], iT_d[:, 0, 0:NT])
            nc.sync.dma_start(wf1[0][:], w_fi1_d[:, 0:512])
            nc.sync.dma_start(bt[:], bias_d[:])
            nc.sync.dma_start(xt_cur[:], tT_d[:, 0:NT])
            nc.sync.dma_start(wt1[0][:], w_ft1_d[:])
            for k in range(1, KI):
                nc.sync.dma_start(x_cur[:, k, :], iT_d[:, k, 0:NT])
                nc.sync.dma_start(wf1[k][:], w_fi1_d[:, 512 * k:512 * (k + 1)])
            for tiles, dram in [(wc1, w_ci1_d), (wc2, w_ct1_d),
                                (wV, w_V_d), (wT, w_T_d)]:
                for j, wtile in enumerate(tiles):
                    nc.sync.dma_start(wtile[:], dram[:, 512 * j:512 * (j + 1)])

            def layer(wtiles, xs, htag, n, bcol, func, K):
                """out[m] = func(sum_k wtiles[k][:,m].T @ xs[k] + bias), m-outer."""
                outs = []
                for m in range(4):
                    ps = pspool.tile([128, NT], f32, tag="ps", name=f"ps_{htag}_{n}_{m}")
                    for k in range(K):
                        nc.tensor.matmul(
                            ps[:],
                            wtiles[k][:, m * 128:(m + 1) * 128],
                            xs[k],
                            start=(k == 0),
                            stop=(k == K - 1),
                        )
                    h = hpool.tile([128, NT], f16, tag=htag, name=f"{htag}_{n}_{m}")
                    nc.scalar.activation(h[:], ps[:], func, bias=bt[:, bcol + m:bcol + m + 1])
                    outs.append(h)
                return outs

            def cat_layer(wtiles, xs_a, xs_b, n, bcol, oname, orow0):
                """out[m] = sum_k w[k].T@xs_a[k] + sum_k w[4+k].T@xs_b[k] + bias; store f32."""
                for m in range(4):
                    ps = pspool.tile([128, NT], f32, tag="ps", name=f"ps_{oname}_{n}_{m}")
                    for k in range(4):
                        nc.tensor.matmul(
                            ps[:], wtiles[k][:, m * 128:(m + 1) * 128], xs_a[k],
                            start=(k == 0), stop=False,
                        )
                    for k in range(4):
                        nc.tensor.matmul(
                            ps[:], wtiles[4 + k][:, m * 128:(m + 1) * 128], xs_b[k],
                            start=False, stop=(k == 3),
                        )
                    o = opool.tile([128, NT], f32, tag="o", name=f"o{oname}_{n}_{m}")
                    nc.scalar.activation(o[:], ps[:], Ident, bias=bt[:, bcol + m:bcol + m + 1])
                    nc.gpsimd.dma_start(
                        out_d[orow0 + 128 * m:orow0 + 128 * (m + 1), n * NT:(n + 1) * NT],
                        o[:],
                    )

            for n in range(ntiles):
                c0 = n * NT
                xs_i = [x_cur[:, k, :] for k in range(KI)]

                # ---- i_ = relu(i @ fi1.T + b) ----
                i_ = layer(wf1, xs_i, "i_", n, 0, Relu, KI)

                # prefetch next tile's inputs on the gpsimd queue
                if n + 1 < ntiles:
                    x_nxt = xpool.tile([128, KI, NT], f16, tag="x", name=f"x_{n + 1}")
                    xt_nxt = xpool.tile([128, NT], f16, tag="xt", name=f"xt_{n + 1}")
                    nc.gpsimd.dma_start(x_nxt[:, 0:8, :], iT_d[:, 0:8, c0 + NT:c0 + 2 * NT])
                    nc.gpsimd.dma_start(x_nxt[:, 8:KI, :], iT_d[:, 8:KI, c0 + NT:c0 + 2 * NT])
                    nc.gpsimd.dma_start(xt_nxt[:], tT_d[:, c0 + NT:c0 + 2 * NT])

                # ---- t_ = relu(t @ ft1.T + b) ----
                t_ = layer(wt1, [xt_cur[:]], "t_", n, 4, Relu, 1)
                # ---- v1 = relu(i_ @ ci1.T + b) ----
                v1 = layer(wc1, [h[:] for h in i_], "v1", n, 8, Relu, 4)
                # ---- v2 = relu(t_ @ ct1.T + b) ----
                v2 = layer(wc2, [h[:] for h in t_], "v2", n, 12, Relu, 4)
                # ---- V = [v1, i_] @ WcatV.T + bcatV ----
                cat_layer(wV, [h[:] for h in v1], [h[:] for h in i_], n, 16, "V", 0)
                # ---- T = [v2, t_] @ WcatT.T + bcatT ----
                cat_layer(wT, [h[:] for h in v2], [h[:] for h in t_], n, 20, "T", HID)

                if n + 1 < ntiles:
                    x_cur = x_nxt
                    xt_cur = xt_nxt

    nc.compile()
    return nc


def _host_pack(inp: dict):
    f8 = np.float64
    fi1_w, fi1_b = inp["fi1_w"], inp["fi1_b"]
    ft1_w, ft1_b = inp["ft1_w"], inp["ft1_b"]
    ci1_w, ci1_b = inp["ci1_w"], inp["ci1_b"]
    ct1_w, ct1_b = inp["ct1_w"], inp["ct1_b"]

    def fold(wv, bv, wo, bo, f_w, f_b):
        Wvo = wo.astype(f8) @ wv.astype(f8)
        bvo = wo.astype(f8) @ bv.astype(f8) + bo.astype(f8)
        Wcat = np.concatenate([f_w.astype(f8) @ Wvo, f_w.astype(f8)], axis=1)
        bcat = f_w.astype(f8) @ bvo + f_b.astype(f8)
        return Wcat.astype(np.float32), bcat.astype(np.float32)

    WcatV, bcatV = fold(inp["aV_wv"], inp["aV_bv"], inp["aV_wo"], inp["aV_bo"],
                        inp["fi2_w"], inp["fi2_b"])
    WcatT, bcatT = fold(inp["aT_wv"], inp["aT_bv"], inp["aT_wo"], inp["aT_bo"],
                        inp["ft2_w"], inp["ft2_b"])

    weights = {
        "w_fi1": _pack_blocks(np.ascontiguousarray(fi1_w.T), 16, 4),
        "w_ft1": _pack_blocks(np.ascontiguousarray(ft1_w.T), 1, 4),
        "w_ci1": _pack_blocks(np.ascontiguousarray(ci1_w.T), 4, 4),
        "w_ct1": _pack_blocks(np.ascontiguousarray(ct1_w.T), 4, 4),
        "w_V": _pack_blocks(np.ascontiguousarray(WcatV.T), 8, 4),
        "w_T": _pack_blocks(np.ascontiguousarray(WcatT.T), 8, 4),
    }
    cols = []
    for b in (fi1_b, ft1_b, ci1_b, ct1_b, bcatV, bcatT):
        for m in range(4):
            cols.append(b[128 * m:128 * (m + 1)])
    weights["bias"] = np.ascontiguousarray(np.stack(cols, axis=1), dtype=np.float32)
    return weights


def make_in_maps(inputs: dict):
    """Full inputs -> per-core input dicts (shard batch, replicate weights)."""
    i = np.asarray(inputs["i"], dtype=np.float32)
    t = np.asarray(inputs["t"], dtype=np.float32)
    weights = _host_pack(inputs)
    i16 = i.astype(np.float16)
    t16 = t.astype(np.float16)
    in_maps = []
    for c in range(NCORES):
        sl = slice(c * BS, (c + 1) * BS)
        m = dict(weights)
        # [128, KI, bs] with [p, k, c] = i[c, 128k + p]
        m["iT"] = np.ascontiguousarray(
            i16[sl].T.reshape(KI, 128, BS).transpose(1, 0, 2))
        m["tT"] = np.ascontiguousarray(t16[sl].T)
        in_maps.append(m)
    return in_maps


def kernel(**inputs) -> np.ndarray:
    from concourse import bass_utils

    if "nc" not in _CACHE:
        _CACHE["nc"] = _build_nc(BS)
    nc = _CACHE["nc"]

    in_maps = make_in_maps(inputs)
    res = bass_utils.run_bass_kernel_spmd(nc, in_maps, core_ids=list(range(NCORES)))

    out = np.empty((B, 2 * HID), dtype=np.float32)
    for c in range(NCORES):
        out[c * BS:(c + 1) * BS] = res.results[c]["outT"].T
    return out


# revision 5
# speedup vs baseline: 1.0785x; 1.0756x over previous
"""Trainium2 Bass kernel for nn_CMFA (dense_transformer, seq_len=1 cross-attention).

Math notes (exact simplifications vs the reference):
  - softmax over a single key is exactly 1.0, so the attention output is
    exactly the v-projection: mha(q,k,v) = (v @ Wv.T + bv) @ Wo.T + bo.
    The q/k projections never influence the output.
  - Wv -> Wo -> fi2 is a linear chain (no nonlinearity), so it is folded on
    the host:  V = [v1, i_] @ Wcat.T + bcat  with
      Wcat = [fi2 @ (Wo @ Wv), fi2],  bcat = fi2 @ (Wo @ bv + bo) + fi2_b
    (the i_ column block carries the residual through fi2).

Precision: matmul operands (inputs, weights, intermediate activations) are
fp16; PSUM accumulation, biases and the final output are fp32. fp16 keeps
the PE at 1 cycle/row (same as f32r at 512-wide moving dim) but halves the
stationary-weight load so it hides under the previous matmul's streaming
window (f32r pays ~25ns per matmul for it), and halves HBM traffic.
Measured end-to-end error vs the f32 reference is ~5e-4 (gate is 2e-2).

Device layout: activations are feature-major ("transposed", [feat, batch]) so
every matmul contracts over the partition dim and every DMA is contiguous.
The host pre-transposes the batch shards of i/t and transposes the output
back. Pure data parallel across 8 cores; weights replicated.

Schedule notes (all from trace measurements):
  - All loads ride the sync queue in program order: each DMA issue costs
    ~610ns of sequencer time, so tile-0 x / fi1-weight chunks are grouped
    (1/2/3/4/6) to stay ahead of the PE's 853ns-per-chunk consumption, and
    prefetches are issued behind the preamble so they cannot steal DMA
    bandwidth from startup-critical transfers.
  - Output stores issue from the scalar queue (hardware DGE, and the act
    that produces the tile runs there, so no cross-engine hop); the gpsimd
    queue's software DGE costs ~1us per store and serializes the tail.
  - The PE drops to half clock for ~3us after any idle gap, so a dozen
    warm-up matmuls on a memset tile bridge the DMA startup window and the
    real stream enters at full clock.
  - t-branch first (ft1 gates on only 192KB of input), then fi1 (k-outer:
    4 matmuls per arriving x chunk), then ct1/ci1 (hides the i_ activation
    latency), then the two folded output layers.
"""

import numpy as np

B, IMG, TAB, HID = 32768, 2048, 128, 512
NCORES = 8
BS = B // NCORES  # rows per core
NT = 512          # batch-tile (matmul moving/free dim)
KI = IMG // 128   # 16 contraction chunks for fi1
NWARM = 12        # PE p-state warm-up matmuls

_CACHE = {}


def _pack_blocks(WT: np.ndarray, K: int, M: int) -> np.ndarray:
    """[K*128, M*128] -> [128, K, M*128] with [p, k, m*128+j] = WT[k*128+p, m*128+j]."""
    out = WT.reshape(K, 128, M * 128).transpose(1, 0, 2)
    return np.ascontiguousarray(out, dtype=np.float16)


def _build_nc(bs: int):
    import concourse.bass as bass
    import concourse.tile as tile
    from concourse import bacc, mybir

    f32 = mybir.dt.float32
    f16 = mybir.dt.float16
    Relu = mybir.ActivationFunctionType.Relu
    Ident = mybir.ActivationFunctionType.Identity
    ntiles = bs // NT

    nc = bacc.Bacc("TRN2", target_bir_lowering=False, debug=False)

    iT_d = nc.dram_tensor("iT", [128, KI, bs], f16, kind="ExternalInput").ap()
    tT_d = nc.dram_tensor("tT", [TAB, bs], f16, kind="ExternalInput").ap()
    w_fi1_d = nc.dram_tensor("w_fi1", [128, KI, 512], f16, kind="ExternalInput").ap()
    w_ft1_d = nc.dram_tensor("w_ft1", [128, 1, 512], f16, kind="ExternalInput").ap()
    w_ci1_d = nc.dram_tensor("w_ci1", [128, 4, 512], f16, kind="ExternalInput").ap()
    w_ct1_d = nc.dram_tensor("w_ct1", [128, 4, 512], f16, kind="ExternalInput").ap()
    w_V_d = nc.dram_tensor("w_V", [128, 8, 512], f16, kind="ExternalInput").ap()
    w_T_d = nc.dram_tensor("w_T", [128, 8, 512], f16, kind="ExternalInput").ap()
    bias_d = nc.dram_tensor("bias", [128, 24], f32, kind="ExternalInput").ap()
    out_d = nc.dram_tensor("outT", [2 * HID, bs], f32, kind="ExternalOutput").ap()

    # x chunk groups for tile 0: sized so grouped DMA issues stay ahead of
    # the PE eating 4 matmuls (853ns) per chunk.
    XGRP = [(0, 1), (1, 3), (3, 6), (6, 10), (10, 16)]

    with tile.TileContext(nc) as tc:
        with (
            tc.tile_pool(name="w", bufs=1) as wpool,
            tc.tile_pool(name="x", bufs=3) as xpool,
            tc.tile_pool(name="h", bufs=8) as hpool,
            tc.tile_pool(name="o", bufs=8) as opool,
            tc.tile_pool(name="ps", bufs=8, space="PSUM") as pspool,
        ):
            wf1 = wpool.tile([128, KI, 512], f16, name="w_fi1_t")
            wt1 = wpool.tile([128, 1, 512], f16, name="w_ft1_t")
            wc1 = wpool.tile([128, 4, 512], f16, name="w_ci1_t")
            wc2 = wpool.tile([128, 4, 512], f16, name="w_ct1_t")
            wV = wpool.tile([128, 8, 512], f16, name="w_V_t")
            wT = wpool.tile([128, 8, 512], f16, name="w_T_t")
            bt = wpool.tile([128, 24], f32, name="bias_t")
            warm = wpool.tile([128, NT], f16, name="warm")

            # ---- PE warm-up: bridge the DMA startup window at rising clock ----
            nc.vector.memset(warm[:], 0.0)
            wps = pspool.tile([128, NT], f32, tag="ps", name="warm_ps")
            for _ in range(NWARM):
                nc.tensor.matmul(wps[:], warm[:, 0:128], warm[:], start=True, stop=True)

            # ---- preamble loads (sync queue, in consumption order) ----
            x_cur = xpool.tile([128, KI, NT], f16, tag="x", name="x_0")
            xt_cur = xpool.tile([128, NT], f16, tag="xt", name="xt_0")
            nc.sync.dma_start(xt_cur[:], tT_d[:, 0:NT])
            nc.sync.dma_start(wt1[:], w_ft1_d[:])
            nc.sync.dma_start(bt[:], bias_d[:])
            for a, b in XGRP:
                nc.sync.dma_start(x_cur[:, a:b, :], iT_d[:, a:b, 0:NT])
                nc.sync.dma_start(wf1[:, a:b, :], w_fi1_d[:, a:b, :])
            nc.sync.dma_start(wc2[:], w_ct1_d[:])
            nc.sync.dma_start(wc1[:], w_ci1_d[:])
            nc.sync.dma_start(wV[:, 0:4, :], w_V_d[:, 0:4, :])
            nc.sync.dma_start(wV[:, 4:8, :], w_V_d[:, 4:8, :])
            nc.sync.dma_start(wT[:, 0:4, :], w_T_d[:, 0:4, :])
            nc.sync.dma_start(wT[:, 4:8, :], w_T_d[:, 4:8, :])

            def act(ps, htag, n, m, bcol, func):
                h = hpool.tile([128, NT], f16, tag=htag, name=f"{htag}_{n}_{m}")
                nc.scalar.activation(h[:], ps[:], func, bias=bt[:, bcol + m:bcol + m + 1])
                return h

            def layer_k_outer(wt, xs, htag, n, bcol, K):
                """All 4 output blocks accumulate in parallel, k outer: 4
                matmuls per input chunk k (rate-matches chunked DMA arrival)."""
                ps = [pspool.tile([128, NT], f32, tag="ps", name=f"ps_{htag}_{n}_{m}")
                      for m in range(4)]
                for k in range(K):
                    for m in range(4):
                        nc.tensor.matmul(ps[m][:], wt[:, k, m * 128:(m + 1) * 128],
                                         xs[k], start=(k == 0), stop=(k == K - 1))
                return [act(ps[m], htag, n, m, bcol, Relu) for m in range(4)]

            def layer_m_outer(wt, xs, htag, n, bcol, K):
                """m outer: each PSUM bank closes after its k loop and drains
                on the scalar engine while the PE works on the next block."""
                outs = []
                for m in range(4):
                    ps = pspool.tile([128, NT], f32, tag="ps", name=f"ps_{htag}_{n}_{m}")
                    for k in range(K):
                        nc.tensor.matmul(ps[:], wt[:, k, m * 128:(m + 1) * 128],
                                         xs[k], start=(k == 0), stop=(k == K - 1))
                    outs.append(act(ps, htag, n, m, bcol, Relu))
                return outs

            def cat_layer(wt, xs_a, xs_b, n, bcol, oname, orow0):
                """out[m] = sum_k w[k].T@xs_a[k] + w[4+k].T@xs_b[k] + bias; f32 store."""
                for m in range(4):
                    ps = pspool.tile([128, NT], f32, tag="ps", name=f"ps_{oname}_{n}_{m}")
                    for k in range(4):
                        nc.tensor.matmul(ps[:], wt[:, k, m * 128:(m + 1) * 128],
                                         xs_a[k], start=(k == 0), stop=False)
                    for k in range(4):
                        nc.tensor.matmul(ps[:], wt[:, 4 + k, m * 128:(m + 1) * 128],
                                         xs_b[k], start=False, stop=(k == 3))
                    o = opool.tile([128, NT], f32, tag="o", name=f"o{oname}_{n}_{m}")
                    nc.scalar.activation(o[:], ps[:], Ident,
                                         bias=bt[:, bcol + m:bcol + m + 1])
                    nc.scalar.dma_start(
                        out_d[orow0 + 128 * m:orow0 + 128 * (m + 1), n * NT:(n + 1) * NT],
                        o[:],
                    )

            for n in range(ntiles):
                c0 = n * NT
                xs_i = [x_cur[:, k, :] for k in range(KI)]

                # ---- t_ = relu(t @ ft1.T + b): gates on only 192KB of input ----
                t_ = layer_m_outer(wt1, [xt_cur[:]], "t_", n, 4, 1)
                # ---- i_ = relu(i @ fi1.T + b) ----
                i_ = layer_k_outer(wf1, xs_i, "i_", n, 0, KI)

                # prefetch next tile's inputs (behind the preamble on sync)
                if n + 1 < ntiles:
                    x_nxt = xpool.tile([128, KI, NT], f16, tag="x", name=f"x_{n + 1}")
                    xt_nxt = xpool.tile([128, NT], f16, tag="xt", name=f"xt_{n + 1}")
                    nc.sync.dma_start(x_nxt[:, 0:8, :], iT_d[:, 0:8, c0 + NT:c0 + 2 * NT])
                    nc.sync.dma_start(x_nxt[:, 8:KI, :], iT_d[:, 8:KI, c0 + NT:c0 + 2 * NT])
                    nc.sync.dma_start(xt_nxt[:], tT_d[:, c0 + NT:c0 + 2 * NT])

                # ---- v2 = relu(t_ @ ct1.T + b): fills the i_ activation latency ----
                v2 = layer_m_outer(wc2, [h[:] for h in t_], "v2", n, 12, 4)
                # ---- v1 = relu(i_ @ ci1.T + b) ----
                v1 = layer_m_outer(wc1, [h[:] for h in i_], "v1", n, 8, 4)
                # ---- V = [v1, i_] @ WcatV.T + bcatV ----
                cat_layer(wV, [h[:] for h in v1], [h[:] for h in i_], n, 16, "V", 0)
                # ---- T = [v2, t_] @ WcatT.T + bcatT ----
                cat_layer(wT, [h[:] for h in v2], [h[:] for h in t_], n, 20, "T", HID)

                if n + 1 < ntiles:
                    x_cur = x_nxt
                    xt_cur = xt_nxt

    nc.compile()
    return nc


def _host_pack(inp: dict):
    f8 = np.float64
    fi1_w, fi1_b = inp["fi1_w"], inp["fi1_b"]
    ft1_w, ft1_b = inp["ft1_w"], inp["ft1_b"]
    ci1_w, ci1_b = inp["ci1_w"], inp["ci1_b"]
    ct1_w, ct1_b = inp["ct1_w"], inp["ct1_b"]

    def fold(wv, bv, wo, bo, f_w, f_b):
        Wvo = wo.astype(f8) @ wv.astype(f8)
        bvo = wo.astype(f8) @ bv.astype(f8) + bo.astype(f8)
        Wcat = np.concatenate([f_w.astype(f8) @ Wvo, f_w.astype(f8)], axis=1)
        bcat = f_w.astype(f8) @ bvo + f_b.astype(f8)
        return Wcat.astype(np.float32), bcat.astype(np.float32)

    WcatV, bcatV = fold(inp["aV_wv"], inp["aV_bv"], inp["aV_wo"], inp["aV_bo"],
                        inp["fi2_w"], inp["fi2_b"])
    WcatT, bcatT = fold(inp["aT_wv"], inp["aT_bv"], inp["aT_wo"], inp["aT_bo"],
                        inp["ft2_w"], inp["ft2_b"])

    weights = {
        "w_fi1": _pack_blocks(np.ascontiguousarray(fi1_w.T), 16, 4),
        "w_ft1": _pack_blocks(np.ascontiguousarray(ft1_w.T), 1, 4),
        "w_ci1": _pack_blocks(np.ascontiguousarray(ci1_w.T), 4, 4),
        "w_ct1": _pack_blocks(np.ascontiguousarray(ct1_w.T), 4, 4),
        "w_V": _pack_blocks(np.ascontiguousarray(WcatV.T), 8, 4),
        "w_T": _pack_blocks(np.ascontiguousarray(WcatT.T), 8, 4),
    }
    cols = []
    for b in (fi1_b, ft1_b, ci1_b, ct1_b, bcatV, bcatT):
        for m in range(4):
            cols.append(b[128 * m:128 * (m + 1)])
    weights["bias"] = np.ascontiguousarray(np.stack(cols, axis=1), dtype=np.float32)
    return weights


def make_in_maps(inputs: dict):
    """Full inputs -> per-core input dicts (shard batch, replicate weights)."""
    i = np.asarray(inputs["i"], dtype=np.float32)
    t = np.asarray(inputs["t"], dtype=np.float32)
    weights = _host_pack(inputs)
    i16 = i.astype(np.float16)
    t16 = t.astype(np.float16)
    in_maps = []
    for c in range(NCORES):
        sl = slice(c * BS, (c + 1) * BS)
        m = dict(weights)
        # [128, KI, bs] with [p, k, c] = i[c, 128k + p]
        m["iT"] = np.ascontiguousarray(
            i16[sl].T.reshape(KI, 128, BS).transpose(1, 0, 2))
        m["tT"] = np.ascontiguousarray(t16[sl].T)
        in_maps.append(m)
    return in_maps


def kernel(**inputs) -> np.ndarray:
    from concourse import bass_utils

    if "nc" not in _CACHE:
        _CACHE["nc"] = _build_nc(BS)
    nc = _CACHE["nc"]

    in_maps = make_in_maps(inputs)
    res = bass_utils.run_bass_kernel_spmd(nc, in_maps, core_ids=list(range(NCORES)))

    out = np.empty((B, 2 * HID), dtype=np.float32)
    for c in range(NCORES):
        out[c * BS:(c + 1) * BS] = res.results[c]["outT"].T
    return out


# revision 13
# speedup vs baseline: 1.0796x; 1.0010x over previous
"""Trainium2 Bass kernel for nn_CMFA (dense_transformer, seq_len=1 cross-attention).

Math notes (exact simplifications vs the reference):
  - softmax over a single key is exactly 1.0, so the attention output is
    exactly the v-projection: mha(q,k,v) = (v @ Wv.T + bv) @ Wo.T + bo.
    The q/k projections never influence the output.
  - Wv -> Wo -> fi2 is a linear chain (no nonlinearity), so it is folded on
    the host:  V = [v1, i_] @ Wcat.T + bcat  with
      Wcat = [fi2 @ (Wo @ Wv), fi2],  bcat = fi2 @ (Wo @ bv + bo) + fi2_b
    (the i_ column block carries the residual through fi2).

Precision: matmul operands (inputs, weights, intermediate activations) are
fp16; PSUM accumulation, biases and the final output are fp32. fp16 keeps
the PE at 1 cycle/row (same as f32r at 512-wide moving dim) but halves the
stationary-weight load so it hides under the previous matmul's streaming
window (f32r pays ~25ns per matmul for it), and halves HBM traffic.
Measured end-to-end error vs the f32 reference is ~5e-4 (gate is 2e-2).

Device layout: activations are feature-major ("transposed", [feat, batch]) so
every matmul contracts over the partition dim and every DMA is contiguous.
The host pre-transposes the batch shards of i/t and transposes the output
back. Pure data parallel across 8 cores; weights replicated.

Schedule notes (all from trace measurements):
  - All loads ride the sync queue in program order: each DMA issue costs
    ~610ns of sequencer time, so tile-0 x / fi1-weight chunks are grouped
    (1/2/3/4/6) to stay ahead of the PE's 853ns-per-chunk consumption, and
    prefetches are issued behind the preamble so they cannot steal DMA
    bandwidth from startup-critical transfers.
  - Output stores issue from the scalar queue (hardware DGE, and the act
    that produces the tile runs there, so no cross-engine hop); the gpsimd
    queue's software DGE costs ~1us per store and serializes the tail.
  - The PE drops to half clock for ~3us after any idle gap, so a dozen
    warm-up matmuls on a memset tile bridge the DMA startup window and the
    real stream enters at full clock.
  - t-branch first (ft1 gates on only 192KB of input), then fi1 (k-outer:
    4 matmuls per arriving x chunk), then ct1/ci1 (hides the i_ activation
    latency), then the two folded output layers.
"""

import numpy as np

B, IMG, TAB, HID = 32768, 2048, 128, 512
NCORES = 8
BS = B // NCORES  # rows per core
NT = 512          # batch-tile (matmul moving/free dim)
KI = IMG // 128   # 16 contraction chunks for fi1
XALL = KI + 1     # + the t chunk, packed as chunk 16 of the same tile
NWARM = 5         # PE p-state warm-up matmuls

_CACHE = {}


def _pack_blocks(WT: np.ndarray, K: int, M: int) -> np.ndarray:
    """[K*128, M*128] -> [128, K, M*128] with [p, k, m*128+j] = WT[k*128+p, m*128+j]."""
    out = WT.reshape(K, 128, M * 128).transpose(1, 0, 2)
    return np.ascontiguousarray(out, dtype=np.float16)


def _build_nc(bs: int):
    import concourse.bass as bass
    import concourse.tile as tile
    from concourse import bacc, mybir

    f32 = mybir.dt.float32
    f16 = mybir.dt.float16
    Relu = mybir.ActivationFunctionType.Relu
    Ident = mybir.ActivationFunctionType.Identity
    ntiles = bs // NT

    nc = bacc.Bacc("TRN2", target_bir_lowering=False, debug=False)

    # tile-major input layout: per batch-tile, all 17 chunks contiguous per
    # partition (16KB lines -> large DMA descriptors, one prefetch per tile)
    iT_d = nc.dram_tensor("iT", [bs // NT, 128, XALL, NT], f16,
                          kind="ExternalInput").ap()
    w_fi1_d = nc.dram_tensor("w_fi1", [128, KI, 512], f16, kind="ExternalInput").ap()
    w_ft1_d = nc.dram_tensor("w_ft1", [128, 1, 512], f16, kind="ExternalInput").ap()
    w_ci1_d = nc.dram_tensor("w_ci1", [128, 4, 512], f16, kind="ExternalInput").ap()
    w_ct1_d = nc.dram_tensor("w_ct1", [128, 4, 512], f16, kind="ExternalInput").ap()
    w_V_d = nc.dram_tensor("w_V", [128, 8, 512], f16, kind="ExternalInput").ap()
    w_T_d = nc.dram_tensor("w_T", [128, 8, 512], f16, kind="ExternalInput").ap()
    bias_d = nc.dram_tensor("bias", [128, 24], f32, kind="ExternalInput").ap()
    out_d = nc.dram_tensor("outT", [2 * HID, bs], f32, kind="ExternalOutput").ap()

    # x chunk groups for tile 0: sized so grouped DMA issues stay ahead of
    # the PE eating 4 matmuls (853ns) per chunk, finer at the back where the
    # cumulative transfer time approaches the PE's consumption schedule.
    XGRP = [(0, 1), (1, 3), (3, 6), (6, 9), (9, 12), (12, 14), (14, 16)]

    with tile.TileContext(nc) as tc:
        with (
            tc.tile_pool(name="w", bufs=1) as wpool,
            tc.tile_pool(name="x", bufs=3) as xpool,
            tc.tile_pool(name="h", bufs=8) as hpool,
            tc.tile_pool(name="o", bufs=8) as opool,
            tc.tile_pool(name="ps", bufs=8, space="PSUM") as pspool,
        ):
            wf1 = wpool.tile([128, KI, 512], f16, name="w_fi1_t")
            wt1 = wpool.tile([128, 1, 512], f16, name="w_ft1_t")
            wc1 = wpool.tile([128, 4, 512], f16, name="w_ci1_t")
            wc2 = wpool.tile([128, 4, 512], f16, name="w_ct1_t")
            wV = wpool.tile([128, 8, 512], f16, name="w_V_t")
            wT = wpool.tile([128, 8, 512], f16, name="w_T_t")
            bt = wpool.tile([128, 24], f32, name="bias_t")
            warm = wpool.tile([128, NT], f16, name="warm")

            # ---- PE warm-up: bridge the DMA startup window at rising clock ----
            nc.vector.memset(warm[:], 0.0)
            wps = pspool.tile([128, NT], f32, tag="ps", name="warm_ps")
            for _ in range(NWARM):
                nc.tensor.matmul(wps[:], warm[:, 0:128], warm[:], start=True, stop=True)

            # ---- preamble loads (sync queue, in consumption order) ----
            x_cur = xpool.tile([128, XALL, NT], f16, tag="x", name="x_0")
            nc.sync.dma_start(bt[:], bias_d[:])
            nc.sync.dma_start(x_cur[:, KI, :], iT_d[0, :, KI, :])
            nc.sync.dma_start(wt1[:], w_ft1_d[:])
            for a, b in XGRP:
                nc.sync.dma_start(x_cur[:, a:b, :], iT_d[0, :, a:b, :])
                nc.sync.dma_start(wf1[:, a:b, :], w_fi1_d[:, a:b, :])
            nc.sync.dma_start(wc2[:], w_ct1_d[:])
            nc.sync.dma_start(wc1[:], w_ci1_d[:])
            nc.sync.dma_start(wV[:, 0:4, :], w_V_d[:, 0:4, :])
            nc.sync.dma_start(wV[:, 4:8, :], w_V_d[:, 4:8, :])
            nc.sync.dma_start(wT[:, 0:4, :], w_T_d[:, 0:4, :])
            nc.sync.dma_start(wT[:, 4:8, :], w_T_d[:, 4:8, :])

            def act(ps, htag, n, m, bcol, func):
                h = hpool.tile([128, NT], f16, tag=htag, name=f"{htag}_{n}_{m}")
                nc.scalar.activation(h[:], ps[:], func, bias=bt[:, bcol + m:bcol + m + 1])
                return h

            def layer_k_outer(wt, xs, htag, n, bcol, K):
                """All 4 output blocks accumulate in parallel, k outer: 4
                matmuls per input chunk k (rate-matches chunked DMA arrival)."""
                ps = [pspool.tile([128, NT], f32, tag="ps", name=f"ps_{htag}_{n}_{m}")
                      for m in range(4)]
                for k in range(K):
                    for m in range(4):
                        nc.tensor.matmul(ps[m][:], wt[:, k, m * 128:(m + 1) * 128],
                                         xs[k], start=(k == 0), stop=(k == K - 1))
                return [act(ps[m], htag, n, m, bcol, Relu) for m in range(4)]

            def layer_m_outer(wt, xs, htag, n, bcol, K):
                """m outer: each PSUM bank closes after its k loop and drains
                on the scalar engine while the PE works on the next block."""
                outs = []
                for m in range(4):
                    ps = pspool.tile([128, NT], f32, tag="ps", name=f"ps_{htag}_{n}_{m}")
                    for k in range(K):
                        nc.tensor.matmul(ps[:], wt[:, k, m * 128:(m + 1) * 128],
                                         xs[k], start=(k == 0), stop=(k == K - 1))
                    outs.append(act(ps, htag, n, m, bcol, Relu))
                return outs

            def cat_layer(wt, xs_a, xs_b, n, bcol, oname, orow0, split_last=False):
                """out[m] = sum_k w[k].T@xs_a[k] + w[4+k].T@xs_b[k] + bias; f32 store.

                split_last: compute the final m in two column halves so the
                kernel's last act+store chain moves half the data (tail cut).
                """
                for m in range(4):
                    ps = pspool.tile([128, NT], f32, tag="ps", name=f"ps_{oname}_{n}_{m}")
                    halves = [(0, NT)]
                    if split_last and m == 3:
                        halves = [(0, NT // 2), (NT // 2, NT)]
                    for c0, c1 in halves:
                        for k in range(4):
                            nc.tensor.matmul(ps[:, c0:c1], wt[:, k, m * 128:(m + 1) * 128],
                                             xs_a[k][:, c0:c1], start=(k == 0), stop=False)
                        for k in range(4):
                            nc.tensor.matmul(ps[:, c0:c1], wt[:, 4 + k, m * 128:(m + 1) * 128],
                                             xs_b[k][:, c0:c1], start=False, stop=(k == 3))
                        o = opool.tile([128, NT], f32, tag="o", name=f"o{oname}_{n}_{m}_{c0}")
                        nc.scalar.activation(o[:, 0:c1 - c0], ps[:, c0:c1], Ident,
                                             bias=bt[:, bcol + m:bcol + m + 1])
                        nc.scalar.dma_start(
                            out_d[orow0 + 128 * m:orow0 + 128 * (m + 1),
                                  n * NT + c0:n * NT + c1],
                            o[:, 0:c1 - c0],
                        )

            for n in range(ntiles):
                xs_i = [x_cur[:, k, :] for k in range(KI)]
                last = n + 1 == ntiles

                # ---- t_ = relu(t @ ft1.T + b): gates on only 256KB of input ----
                t_ = layer_m_outer(wt1, [x_cur[:, KI, :]], "t_", n, 4, 1)
                # ---- i_ = relu(i @ fi1.T + b) ----
                i_ = layer_k_outer(wf1, xs_i, "i_", n, 0, KI)

                # prefetch next tile's inputs (behind the preamble on sync)
                if not last:
                    x_nxt = xpool.tile([128, XALL, NT], f16, tag="x", name=f"x_{n + 1}")
                    nc.sync.dma_start(x_nxt[:], iT_d[n + 1])

                # ---- v2 = relu(t_ @ ct1.T + b): fills the i_ activation latency ----
                v2 = layer_m_outer(wc2, [h[:] for h in t_], "v2", n, 12, 4)
                # ---- v1 = relu(i_ @ ci1.T + b) ----
                v1 = layer_m_outer(wc1, [h[:] for h in i_], "v1", n, 8, 4)
                # ---- V = [v1, i_] @ WcatV.T + bcatV ----
                cat_layer(wV, [h[:] for h in v1], [h[:] for h in i_], n, 16, "V", 0)
                # ---- T = [v2, t_] @ WcatT.T + bcatT ----
                cat_layer(wT, [h[:] for h in v2], [h[:] for h in t_], n, 20, "T", HID,
                          split_last=last)

                if not last:
                    x_cur = x_nxt

    nc.compile()
    return nc


def _host_pack(inp: dict):
    f8 = np.float64
    fi1_w, fi1_b = inp["fi1_w"], inp["fi1_b"]
    ft1_w, ft1_b = inp["ft1_w"], inp["ft1_b"]
    ci1_w, ci1_b = inp["ci1_w"], inp["ci1_b"]
    ct1_w, ct1_b = inp["ct1_w"], inp["ct1_b"]

    def fold(wv, bv, wo, bo, f_w, f_b):
        Wvo = wo.astype(f8) @ wv.astype(f8)
        bvo = wo.astype(f8) @ bv.astype(f8) + bo.astype(f8)
        Wcat = np.concatenate([f_w.astype(f8) @ Wvo, f_w.astype(f8)], axis=1)
        bcat = f_w.astype(f8) @ bvo + f_b.astype(f8)
        return Wcat.astype(np.float32), bcat.astype(np.float32)

    WcatV, bcatV = fold(inp["aV_wv"], inp["aV_bv"], inp["aV_wo"], inp["aV_bo"],
                        inp["fi2_w"], inp["fi2_b"])
    WcatT, bcatT = fold(inp["aT_wv"], inp["aT_bv"], inp["aT_wo"], inp["aT_bo"],
                        inp["ft2_w"], inp["ft2_b"])

    weights = {
        "w_fi1": _pack_blocks(np.ascontiguousarray(fi1_w.T), 16, 4),
        "w_ft1": _pack_blocks(np.ascontiguousarray(ft1_w.T), 1, 4),
        "w_ci1": _pack_blocks(np.ascontiguousarray(ci1_w.T), 4, 4),
        "w_ct1": _pack_blocks(np.ascontiguousarray(ct1_w.T), 4, 4),
        "w_V": _pack_blocks(np.ascontiguousarray(WcatV.T), 8, 4),
        "w_T": _pack_blocks(np.ascontiguousarray(WcatT.T), 8, 4),
    }
    cols = []
    for b in (fi1_b, ft1_b, ci1_b, ct1_b, bcatV, bcatT):
        for m in range(4):
            cols.append(b[128 * m:128 * (m + 1)])
    weights["bias"] = np.ascontiguousarray(np.stack(cols, axis=1), dtype=np.float32)
    return weights


def make_in_maps(inputs: dict):
    """Full inputs -> per-core input dicts (shard batch, replicate weights)."""
    i = np.asarray(inputs["i"], dtype=np.float32)
    t = np.asarray(inputs["t"], dtype=np.float32)
    weights = _host_pack(inputs)
    i16 = i.astype(np.float16)
    t16 = t.astype(np.float16)
    ntiles = BS // NT
    in_maps = []
    for c in range(NCORES):
        sl = slice(c * BS, (c + 1) * BS)
        m = dict(weights)
        # [ntiles, 128, XALL, NT]: batch-tile major; chunk k<16 holds
        # i[n*NT+j, 128k+p], chunk 16 holds t[n*NT+j, p].
        xi = i16[sl].T.reshape(KI, 128, ntiles, NT)   # [k, p, n, j]
        xt = t16[sl].T.reshape(TAB, ntiles, NT)       # [p, n, j]
        full = np.empty((ntiles, 128, XALL, NT), dtype=np.float16)
        full[:, :, :KI, :] = xi.transpose(2, 1, 0, 3)
        full[:, :, KI, :] = xt.transpose(1, 0, 2)
        m["iT"] = full
        in_maps.append(m)
    return in_maps


def kernel(**inputs) -> np.ndarray:
    from concourse import bass_utils

    if "nc" not in _CACHE:
        _CACHE["nc"] = _build_nc(BS)
    nc = _CACHE["nc"]

    in_maps = make_in_maps(inputs)
    res = bass_utils.run_bass_kernel_spmd(nc, in_maps, core_ids=list(range(NCORES)))

    out = np.empty((B, 2 * HID), dtype=np.float32)
    for c in range(NCORES):
        out[c * BS:(c + 1) * BS] = res.results[c]["outT"].T
    return out
